# revision 13
# baseline (speedup 1.0000x reference)
"""Collaborative RNN (GRU-style user-state scan + big vocab projection) on 8 trn2 cores.

Strategy
--------
Data-parallel over batch: core c owns batch rows [4c, 4c+4) (512 (b,t) pairs).
Each batch row (128 pairs) is an INDEPENDENT scan, and output chunk r of the
big projection only needs row r's final hidden state.  The program is software
pipelined per row with one row of lookahead:

    scan(0), scan(1), proj(0), scan(2), proj(1), scan(3), proj(2), proj(3)

so every row's scan hides under the previous row's DMA-bound projection.

The scan is restructured by dependency *levels* within each row: pair t
depends only on the previous occurrence of the same user in the same row.
Level 0 (first occurrences) needs no hidden-state input when h0 == 0.
All per-row levels are <=128 wide (single tile).  Biases are folded into the
embedding table P_cat on the host, so activations are pure sigmoid/tanh and
r|z share one fused sigmoid.

Projection: fp16 ws (host-cast, padded to VP=30720), fp16 logits staging
(host upcasts to fp32), 512-wide matmuls into 1024-wide 2-bank PSUM
supertiles (3 bufs), one PSUM->SBUF cast-copy per supertile balanced 6:5
ACT:DVE, 2.5MB output DMAs.  Scan PSUM lives in 2 packed banks.
"""

import sys
import types

import numpy as np

# ---------------------------------------------------------------- constants
B, S, U, H, V = 32, 128, 256, 128, 30001
NC = 8
R = B // NC  # batch rows per core
N = R * S  # 512 output rows (pairs) per core
H2 = 2 * H
P = 128
NCH = N // P  # pair chunks per core == batch rows per core
VP = 30720  # V padded (multiple of STG/PS_W/MM_N); host pads ws with zeros
WS_CHUNK = 7680  # ws free-dim tile width (fp16, 1.97MB per DMA)
STG_CHUNK = 10240  # staging tile width (fp16, 2.5MB per logits DMA)
PS_W = 1024  # PSUM supertile width (2 banks), one copy per supertile
MM_N = 512  # moving free dim per matmul (PSUM bank limit in fp32)

TRACE = False  # set by test.py for profiling runs
_LAST_RESULTS = {}  # test.py reads exec_time_ns etc. from here


def _install_ntff_hook():
    """Register the axon NTFF profiling hook (antenv.axon_hooks is a stub in
    this container).  Harmless if the .so lacks the profiling symbols."""
    try:
        import antenv

        if getattr(antenv, "axon_hooks", None) is not None:
            return
        mod = types.ModuleType("antenv.axon_hooks")
        mod._hook = None
        mod.set_axon_ntff_profile_hook = lambda h: setattr(mod, "_hook", h)
        mod.get_axon_ntff_profile_hook = lambda: mod._hook
        sys.modules["antenv.axon_hooks"] = mod
        antenv.axon_hooks = mod
        from trn_agent_boot.trn_boot import _ntff_profile_via_ctypes

        hook = _ntff_profile_via_ctypes("/opt/axon/libaxon_pjrt.so")
        if hook is not None:
            mod.set_axon_ntff_profile_hook(hook)
    except Exception:
        pass


# ---------------------------------------------------------------- host prep
def _fold(a, cols):
    """[cols*128] -> [128, cols] with column j = slice j*128:(j+1)*128."""
    return np.ascontiguousarray(a.reshape(cols, P).T)


def _levels_for_row(users_row):
    """occ/prev per step t within one batch row (row-local indices)."""
    occ = np.zeros(S, np.int32)
    prev = np.full(S, -1, np.int32)
    seen_cnt = {}
    seen_last = {}
    for t in range(S):
        u = int(users_row[t])
        occ[t] = seen_cnt.get(u, 0)
        prev[t] = seen_last.get(u, -1)
        seen_cnt[u] = occ[t] + 1
        seen_last[u] = t
    return occ, prev


def _build_core_data(users, items, h0, with_h0):
    """Per-(core,row) level structure + global padded level sizes.

    Per-core tensors (concatenated across (k, r) to keep DMA count low):
      items_all [P, R]        column r = items of row r
      prev_all  [P, sum_k R*nk[k]]  f32, block (k,r): predecessor index
                (row-local t for k==1; compact index into level k-1 for k>1),
                replicated across partitions, padded with -1
      pk_all    [P, R*(K-1)]  f32, column (k-1)*R+r: row-local pair index of
                level-k slot j (in partition j), padded -1
      idx_all   [P, R*(K-1)]  i32, item id per level slot (padded 0)
      invm_all  [P, R*(K-1)]  f32, 0 where row pair is replaced by level k
    """
    rows = []
    kmax = 1
    for c in range(NC):
        rr = []
        for r in range(R):
            occ, prev = _levels_for_row(users[c * R + r])
            rr.append((occ, prev))
            kmax = max(kmax, int(occ.max()) + 1)
        rows.append(rr)

    nk = [0] * kmax
    for c in range(NC):
        for r in range(R):
            occ, _ = rows[c][r]
            for k in range(1, kmax):
                nk[k] = max(nk[k], int((occ == k).sum()))
    nk = [max(2, n) if k > 0 else 0 for k, n in enumerate(nk)]

    nlev = kmax - 1
    prev_cols = sum(nk[1:]) * R
    per_core = []
    for c in range(NC):
        items_c = items[c * R : (c + 1) * R].reshape(-1).astype(np.int32)
        d = {"items_all": _fold(items_c, NCH)}
        if with_h0:
            users_c = users[c * R : (c + 1) * R].reshape(-1).astype(np.int32)
            local_r = np.repeat(np.arange(R, dtype=np.int32), S)
            d["h0_idx"] = _fold(local_r * U + users_c, NCH)
            d["h0c"] = np.ascontiguousarray(
                h0[c * R : (c + 1) * R].reshape(R * U, H), dtype=np.float32
            )
        if nlev:
            prev_all = np.full((P, prev_cols), -1.0, np.float32)
            pk_all = np.full((P, R * nlev), -1.0, np.float32)
            idx_all = np.zeros((P, R * nlev), np.int32)
            invm_all = np.ones((P, R * nlev), np.float32)
            off = 0
            for k in range(1, kmax):
                n = nk[k]
                for r in range(R):
                    occ, prev = rows[c][r]
                    items_r = items_c[r * S : (r + 1) * S]
                    pk = np.nonzero(occ == k)[0]
                    m = len(pk)
                    col = (k - 1) * R + r
                    if k == 1:
                        prev_v = prev[pk].astype(np.float32)
                    else:
                        prev_pk = np.nonzero(occ == k - 1)[0]
                        pos = {int(p): i for i, p in enumerate(prev_pk)}
                        prev_v = np.array(
                            [pos[int(prev[p])] for p in pk], np.float32
                        )
                    prev_all[:, off + r * n : off + r * n + m] = prev_v[None, :]
                    pk_all[:m, col] = pk
                    idx_all[:m, col] = items_r[pk]
                    invm_all[pk, col] = 0.0
                off += R * n
            d["prev_all"] = prev_all
            d["pk_all"] = pk_all
            d["idx_all"] = idx_all
            d["invm_all"] = invm_all
        per_core.append(d)
    return per_core, kmax, nk


# ---------------------------------------------------------------- device build
def _build_program(kmax, nk, with_h0):
    import concourse.bacc as bacc
    import concourse.mybir as mybir
    import concourse.tile as tile
    from concourse import bass
    from concourse.masks import make_identity

    f32 = mybir.dt.float32
    f16 = mybir.dt.float16
    i32 = mybir.dt.int32
    AF = mybir.ActivationFunctionType
    OP = mybir.AluOpType

    nc = bacc.Bacc(None, target_bir_lowering=False)

    nlev = kmax - 1
    prev_cols = sum(nk[1:]) * R
    # column offset of (k, r) block inside prev_all
    prev_off = {}
    off = 0
    for k in range(1, kmax):
        for r in range(R):
            prev_off[(k, r)] = off + r * nk[k]
        off += R * nk[k]

    # ---- DRAM I/O
    items_all = nc.dram_tensor("items_all", [P, NCH], i32, kind="ExternalInput")
    P_cat = nc.dram_tensor("P_cat", [V, H2 + H], f32, kind="ExternalInput")
    W_ru = nc.dram_tensor("W_ru", [H, H2], f32, kind="ExternalInput")
    W_c = nc.dram_tensor("W_c", [H, H], f32, kind="ExternalInput")
    ws = nc.dram_tensor("ws", [H, VP], f16, kind="ExternalInput")
    logits = nc.dram_tensor("logits", [N, VP], f16, kind="ExternalOutput")
    if nlev:
        prev_all = nc.dram_tensor("prev_all", [P, prev_cols], f32, kind="ExternalInput")
        pk_all = nc.dram_tensor("pk_all", [P, R * nlev], f32, kind="ExternalInput")
        idx_all = nc.dram_tensor("idx_all", [P, R * nlev], i32, kind="ExternalInput")
        invm_all = nc.dram_tensor("invm_all", [P, R * nlev], f32, kind="ExternalInput")
    if with_h0:
        h0_idx = nc.dram_tensor("h0_idx", [P, NCH], i32, kind="ExternalInput")
        h0c = nc.dram_tensor("h0c", [R * U, H], f32, kind="ExternalInput")

    ws_splits = [(v0, min(WS_CHUNK, VP - v0)) for v0 in range(0, VP, WS_CHUNK)]

    with (
        tile.TileContext(nc) as tc,
        tc.tile_pool(name="const", bufs=1) as cpool,
        tc.tile_pool(name="scan", bufs=4) as spool,
        tc.tile_pool(name="scan_ps", bufs=1, space="PSUM") as spsum,
        tc.tile_pool(name="big", bufs=3) as bpool,
        tc.tile_pool(name="big_ps", bufs=3, space="PSUM") as bpsum,
    ):
        # ---------- prefetch: sync queue order = items, lvl data, W, ws bulk
        items_sb = cpool.tile([P, NCH], i32, tag="items_sb")
        nc.sync.dma_start(items_sb[:], items_all[:])
        if nlev:
            prev_sb = cpool.tile([P, prev_cols], f32, tag="prev_sb")
            nc.sync.dma_start(prev_sb[:], prev_all[:])
            pk_sb = cpool.tile([P, R * nlev], f32, tag="pk_sb")
            nc.sync.dma_start(pk_sb[:], pk_all[:])
            idx_sb = cpool.tile([P, R * nlev], i32, tag="idx_sb")
            nc.sync.dma_start(idx_sb[:], idx_all[:])
            invm_sb = cpool.tile([P, R * nlev], f32, tag="invm_sb")
            nc.sync.dma_start(invm_sb[:], invm_all[:])
        if with_h0:
            h0_idx_sb = cpool.tile([P, NCH], i32, tag="h0_idx_sb")
            nc.sync.dma_start(h0_idx_sb[:], h0_idx[:])
        w_ru_sb = cpool.tile([H, H2], f32, tag="w_ru")
        nc.sync.dma_start(w_ru_sb[:], W_ru[:])
        w_c_sb = cpool.tile([H, H], f32, tag="w_c")
        nc.sync.dma_start(w_c_sb[:], W_c[:])
        # bulk ws load LAST on the sync queue (fp16, 4 x ~2MB chunks)
        ws_sb = []
        for i, (v0, w) in enumerate(ws_splits):
            t = cpool.tile([H, w], f16, tag=f"ws{i}", name=f"ws{i}")
            nc.sync.dma_start(t[:], ws[:, v0 : v0 + w])
            ws_sb.append(t)

        # ---------- gpsimd queue: iotas/identity first, then gathers row-major
        ident = cpool.tile([P, P], f32, tag="ident")
        make_identity(nc, ident[:])
        iota_col_i = cpool.tile([P, 1], i32, tag="iota_col_i")
        nc.gpsimd.iota(iota_col_i[:], pattern=[[1, 1]], base=0, channel_multiplier=1)
        iota_col = cpool.tile([P, 1], f32, tag="iota_col")
        nc.vector.tensor_copy(iota_col[:], iota_col_i[:])
        iota_row_i = cpool.tile([P, P], i32, tag="iota_row_i")
        nc.gpsimd.iota(iota_row_i[:], pattern=[[1, P]], base=0, channel_multiplier=0)
        iota_row = cpool.tile([P, P], f32, tag="iota_row")
        nc.vector.tensor_copy(iota_row[:], iota_row_i[:])

        g_cat = []
        g_h0 = []
        lvl_emb = {}
        for r in range(NCH):
            t = spool.tile([P, H2 + H], f32, tag="g_cat", bufs=NCH, name="g_cat")
            nc.gpsimd.indirect_dma_start(
                out=t[:],
                out_offset=None,
                in_=P_cat[:],
                in_offset=bass.IndirectOffsetOnAxis(ap=items_sb[:, r : r + 1], axis=0),
            )
            g_cat.append(t)
            if with_h0:
                g = spool.tile([P, H], f32, tag="g_h0", bufs=NCH, name="g_h0")
                nc.gpsimd.indirect_dma_start(
                    out=g[:],
                    out_offset=None,
                    in_=h0c[:],
                    in_offset=bass.IndirectOffsetOnAxis(
                        ap=h0_idx_sb[:, r : r + 1], axis=0
                    ),
                )
                g_h0.append(g)
            for k in range(1, kmax):
                n = nk[k]
                col = (k - 1) * R + r
                e_cat = spool.tile(
                    [P, H2 + H], f32, tag="e_cat", bufs=NCH * max(1, nlev),
                    name="e_cat",
                )
                nc.gpsimd.indirect_dma_start(
                    out=e_cat[:n, :],
                    out_offset=None,
                    in_=P_cat[:],
                    in_offset=bass.IndirectOffsetOnAxis(
                        ap=idx_sb[:n, col : col + 1], axis=0
                    ),
                )
                lvl_emb[(k, r)] = e_cat

        # ---------- DVE: one-hot gather/scatter matrices, row-major
        lvl_sg = {}
        lvl_ss = {}
        for r in range(NCH):
            for k in range(1, kmax):
                n = nk[k]
                col = (k - 1) * R + r
                po = prev_off[(k, r)]
                sg = spool.tile(
                    [P, n], f32, tag="sg", bufs=NCH * max(1, nlev), name="sg"
                )
                nc.vector.tensor_scalar(
                    out=sg[:],
                    in0=prev_sb[:, po : po + n],
                    scalar1=iota_col[:, 0:1],
                    scalar2=None,
                    op0=OP.is_equal,
                )
                lvl_sg[(k, r)] = sg
                ss = spool.tile(
                    [P, P], f32, tag="ss", bufs=NCH * max(1, nlev), name="ss"
                )
                nc.vector.tensor_scalar(
                    out=ss[:n, :],
                    in0=iota_row[:n, :],
                    scalar1=pk_sb[:n, col : col + 1],
                    scalar2=None,
                    op0=OP.is_equal,
                )
                lvl_ss[(k, r)] = ss

        # persistent per-row state
        h_nat = [
            cpool.tile([P, H], f32, tag=f"h_nat{r}", name=f"h_nat{r}")
            for r in range(NCH)
        ]
        hT = [
            cpool.tile([H, P], f16, tag=f"hT{r}", name=f"hT{r}")
            for r in range(NCH)
        ]

        # ---------- per-row scan ----------
        # scan PSUM: 2 packed banks.  tagA [H,512]: hp[0:128] tr[128:256]
        # d[256:384] fin[384:512]; tagB [H,512]: r[0:n] z[n:2n] c[2n:3n]
        # (L0 zero-h0 path: z[0:128] c[128:256]).
        def scan_row(r):
            psA = spsum.tile([H, 512], f32, tag="psA", name="psA")
            psB = spsum.tile([H, 512], f32, tag="psB", name="psB")

            if not with_h0:
                # L0: h = (1-z)*c,  z = sigmoid(P_z[i]+b), c = tanh(P_c[i]+b)
                nc.tensor.matmul(
                    psB[:, 0:P], g_cat[r][:, H:H2], ident[:],
                    is_transpose=True, start=True, stop=True,
                )
                nc.tensor.matmul(
                    psB[:, P : 2 * P], g_cat[r][:, H2 : H2 + H], ident[:],
                    is_transpose=True, start=True, stop=True,
                )
                zT = spool.tile([H, P], f32, tag="zT", name="zT")
                nc.scalar.activation(zT[:], psB[:, 0:P], AF.Sigmoid)
                cT = spool.tile([H, P], f32, tag="cT", name="cT")
                nc.scalar.activation(cT[:], psB[:, P : 2 * P], AF.Tanh)
                hT0 = spool.tile([H, P], f32, tag="hT0", name="hT0")
                nc.vector.tensor_mul(hT0[:], zT[:], cT[:])
                nc.vector.tensor_sub(hT0[:], cT[:], hT0[:])
            else:
                # L0 full GRU vs h_prev = h0[user]
                nc.tensor.matmul(
                    psA[:, 0:P], g_h0[r][:], ident[:],
                    is_transpose=True, start=True, stop=True,
                )
                hprevT = spool.tile([H, P], f32, tag="hprevT", name="hprevT")
                nc.vector.tensor_copy(hprevT[:], psA[:, 0:P])
                nc.tensor.matmul(
                    psB[:, 0:P], g_cat[r][:, 0:H], ident[:],
                    is_transpose=True, start=True, stop=False,
                )
                nc.tensor.matmul(
                    psB[:, 0:P], w_ru_sb[:, 0:H], hprevT[:],
                    start=False, stop=True,
                )
                nc.tensor.matmul(
                    psB[:, P : 2 * P], g_cat[r][:, H:H2], ident[:],
                    is_transpose=True, start=True, stop=False,
                )
                nc.tensor.matmul(
                    psB[:, P : 2 * P], w_ru_sb[:, H:H2], hprevT[:],
                    start=False, stop=True,
                )
                rzT = spool.tile([H, 2 * P], f32, tag="rzT", name="rzT")
                nc.scalar.activation(rzT[:], psB[:, 0 : 2 * P], AF.Sigmoid)
                rh = spool.tile([H, P], f32, tag="rh", name="rh")
                nc.vector.tensor_mul(rh[:], rzT[:, 0:P], hprevT[:])
                nc.tensor.matmul(
                    psB[:, 2 * P : 3 * P], g_cat[r][:, H2 : H2 + H], ident[:],
                    is_transpose=True, start=True, stop=False,
                )
                nc.tensor.matmul(
                    psB[:, 2 * P : 3 * P], w_c_sb[:], rh[:],
                    start=False, stop=True,
                )
                cT = spool.tile([H, P], f32, tag="cT", name="cT")
                nc.scalar.activation(cT[:], psB[:, 2 * P : 3 * P], AF.Tanh)
                # h = c + z*(hprev - c)
                hT0 = spool.tile([H, P], f32, tag="hT0", name="hT0")
                nc.vector.tensor_sub(hT0[:], hprevT[:], cT[:])
                nc.vector.tensor_mul(hT0[:], rzT[:, P : 2 * P], hT0[:])
                nc.vector.tensor_add(hT0[:], cT[:], hT0[:])

            # natural layout for level gathers / scatters
            nc.tensor.transpose(psA[:P, P : 2 * P], hT0[:], ident[:])
            nc.vector.tensor_copy(h_nat[r][:], psA[:P, P : 2 * P])

            # ---------- levels 1..kmax-1 (row-local, single tile each)
            hn_prev = None
            for k in range(1, kmax):
                n = nk[k]
                njp = P if k == 1 else nk[k - 1]
                col = (k - 1) * R + r
                e_cat = lvl_emb[(k, r)]
                psl = spsum.tile([H, 512], f32, tag="psA", name="psA")
                psr = spsum.tile([H, 512], f32, tag="psB", name="psB")
                src = h_nat[r] if k == 1 else hn_prev
                nc.tensor.matmul(
                    psl[:, 0:n], src[:njp, :], lvl_sg[(k, r)][:njp, :n],
                    start=True, stop=True,
                )
                hprevT = spool.tile([H, P], f32, tag="hprevT_l", name="hprevT")
                nc.vector.tensor_copy(hprevT[:, :n], psl[:, 0:n])

                nc.tensor.matmul(
                    psr[:, 0:n], e_cat[:n, 0:H], ident[:n, :n],
                    is_transpose=True, start=True, stop=False,
                )
                nc.tensor.matmul(
                    psr[:, 0:n], w_ru_sb[:, 0:H], hprevT[:, :n],
                    start=False, stop=True,
                )
                nc.tensor.matmul(
                    psr[:, n : 2 * n], e_cat[:n, H:H2], ident[:n, :n],
                    is_transpose=True, start=True, stop=False,
                )
                nc.tensor.matmul(
                    psr[:, n : 2 * n], w_ru_sb[:, H:H2], hprevT[:, :n],
                    start=False, stop=True,
                )
                rzT = spool.tile([H, 2 * P], f32, tag="rzT_l", name="rzT")
                nc.scalar.activation(rzT[:, : 2 * n], psr[:, 0 : 2 * n], AF.Sigmoid)
                rh = spool.tile([H, P], f32, tag="rh_l", name="rh")
                nc.vector.tensor_mul(rh[:, :n], rzT[:, :n], hprevT[:, :n])
                nc.tensor.matmul(
                    psr[:, 2 * n : 3 * n], e_cat[:n, H2 : H2 + H], ident[:n, :n],
                    is_transpose=True, start=True, stop=False,
                )
                nc.tensor.matmul(
                    psr[:, 2 * n : 3 * n], w_c_sb[:], rh[:, :n],
                    start=False, stop=True,
                )
                cTl = spool.tile([H, P], f32, tag="cT_l", name="cTl")
                nc.scalar.activation(cTl[:, :n], psr[:, 2 * n : 3 * n], AF.Tanh)
                # h_new = c + z*(hprev - c)
                hnT = spool.tile([H, P], f32, tag="hnT_l", name="hnT")
                nc.vector.tensor_sub(hnT[:, :n], hprevT[:, :n], cTl[:, :n])
                nc.vector.tensor_mul(hnT[:, :n], rzT[:, n : 2 * n], hnT[:, :n])
                nc.vector.tensor_add(hnT[:, :n], cTl[:, :n], hnT[:, :n])

                hn = spool.tile([P, H], f32, tag="hn_nat", name="hn")
                nc.tensor.transpose(psl[:n, P : 2 * P], hnT[:, :n], ident[:H, :H])
                nc.vector.tensor_copy(hn[:n, :], psl[:n, P : 2 * P])

                # scatter back: h_nat = h_nat * invm + ss.T @ hn
                nc.tensor.matmul(
                    psl[:P, 256 : 256 + H], lvl_ss[(k, r)][:n, :], hn[:n, :],
                    start=True, stop=True,
                )
                nc.vector.scalar_tensor_tensor(
                    out=h_nat[r][:],
                    in0=h_nat[r][:],
                    scalar=invm_sb[:, col : col + 1],
                    in1=psl[:P, 256 : 256 + H],
                    op0=OP.mult,
                    op1=OP.add,
                )
                hn_prev = hn

            # final transposed fp16 state for the projection
            psf = spsum.tile([H, 512], f32, tag="psA", name="psA")
            nc.tensor.transpose(psf[:, 384 : 384 + P], h_nat[r][:], ident[:])
            nc.vector.tensor_copy(hT[r][:], psf[:, 384 : 384 + P])

        # ---------- per-row projection ----------
        cp_state = [0]

        def proj_row(r):
            cp = cp_state[0]
            for v0 in range(0, VP, STG_CHUNK):
                stage = bpool.tile([P, STG_CHUNK], f16, tag="stage", name="stage")
                for t0 in range(0, STG_CHUNK, PS_W):
                    o_ps = bpsum.tile([P, PS_W], f32, tag="o_ps", name="o_ps")
                    for m0 in range(0, PS_W, MM_N):
                        wsi, woff = divmod(v0 + t0 + m0, WS_CHUNK)
                        nc.tensor.matmul(
                            o_ps[:, m0 : m0 + MM_N],
                            hT[r][:],
                            ws_sb[wsi][:, woff : woff + MM_N],
                            start=True,
                            stop=True,
                        )
                    # ACT is ~1.2x faster per copy than DVE -> 6:5 split
                    if cp % 11 in (0, 2, 4, 6, 8, 10):
                        nc.scalar.copy(stage[:, t0 : t0 + PS_W], o_ps[:])
                    else:
                        nc.vector.tensor_copy(stage[:, t0 : t0 + PS_W], o_ps[:])
                    cp += 1
                nc.sync.dma_start(
                    logits[r * P : (r + 1) * P, v0 : v0 + STG_CHUNK],
                    stage[:],
                )
            cp_state[0] = cp

        # ---------- software pipeline: 1-row scan lookahead
        scan_row(0)
        scan_row(1)
        proj_row(0)
        scan_row(2)
        proj_row(1)
        scan_row(3)
        proj_row(2)
        proj_row(3)

    nc.finalize()
    return nc


_PROGRAM_CACHE = {}


def kernel(users, items, h0, P_ru, W_ru, b_ru, P_c, W_c, b_c, ws):
    _install_ntff_hook()
    from concourse.bass_utils import run_bass_kernel_spmd

    users = np.asarray(users)
    items = np.asarray(items)
    h0 = np.asarray(h0, dtype=np.float32)
    with_h0 = bool(np.any(h0))

    per_core, kmax, nk = _build_core_data(users, items, h0, with_h0)

    key = (kmax, tuple(nk), with_h0)
    if key not in _PROGRAM_CACHE:
        _PROGRAM_CACHE[key] = _build_program(kmax, nk, with_h0)
    nc = _PROGRAM_CACHE[key]

    # biases folded into the embedding table: activations need no bias input
    P_cat = np.concatenate(
        [
            np.asarray(P_ru, dtype=np.float32)
            + np.asarray(b_ru, dtype=np.float32)[None, :],
            np.asarray(P_c, dtype=np.float32)
            + np.asarray(b_c, dtype=np.float32)[None, :],
        ],
        axis=1,
    )
    ws_pad = np.zeros((H, VP), np.float16)
    ws_pad[:, :V] = np.asarray(ws, dtype=np.float16)
    shared = {
        "P_cat": P_cat,
        "W_ru": np.ascontiguousarray(W_ru, dtype=np.float32),
        "W_c": np.ascontiguousarray(W_c, dtype=np.float32),
        "ws": ws_pad,
    }
    in_maps = [{**shared, **per_core[c]} for c in range(NC)]

    res = run_bass_kernel_spmd(nc, in_maps, core_ids=list(range(NC)), trace=TRACE)
    _LAST_RESULTS["exec_time_ns"] = res.exec_time_ns
    _LAST_RESULTS["mean_exec_time_ns"] = res.mean_exec_time_ns
    _LAST_RESULTS["trace"] = res.instructions_and_trace
    _LAST_RESULTS["profile_json"] = res.profile_json

    out = np.empty((B * S, V), np.float32)
    for c in range(NC):
        out[c * N : (c + 1) * N] = res.results[c]["logits"][:, :V]
    return out


# revision 15
# speedup vs baseline: 1.3736x; 1.3736x over previous
"""Collaborative RNN (GRU-style user-state scan + big vocab projection) on 8 trn2 cores.

Strategy
--------
Data-parallel over batch: core c owns batch rows [4c, 4c+4) (512 (b,t) pairs).
Each core runs the scan for its rows and computes logits for its 512 output
rows over the FULL vocab; host concatenates and upcasts.

Scan: restructured by dependency *levels* (pair (b,t) depends only on the
previous occurrence of the same user in the same batch row), jointly across
the core's 4 rows so each level is one batched set of ops.  Biases are folded
into the embedding table P_cat on the host (activations need no bias input).
The identity matrix is built on DVE (gpsimd make_identity costs ~8.5us of
queue drain); iotas are emitted before the gathers; one-hot gather/scatter
matrices are built just-in-time per level.  All scan-critical DMA loads are
emitted BEFORE the bulk ws load on the sync queue (HWDGE drains FIFO).

Projection: fp16 ws (host-cast, padded to VP=30720), fp16 logits staging
(host upcasts to fp32), 512-wide matmuls into 1024-wide 2-bank PSUM
supertiles (4 bufs), one PSUM->SBUF cast-copy per supertile balanced 6:5
ACT:DVE, 2.5MB output DMAs (~373 GB/s measured on the HWDGE queue).
"""

import sys
import types

import numpy as np

# ---------------------------------------------------------------- constants
B, S, U, H, V = 32, 128, 256, 128, 30001
NC = 8
R = B // NC  # batch rows per core
N = R * S  # 512 output rows (pairs) per core
H2 = 2 * H
P = 128
NCH = N // P  # pair chunks per core
VP = 30720  # V padded (multiple of STG/PS_W/MM_N); host pads ws with zeros
WS_CHUNK = 7680  # ws free-dim tile width (fp16, 1.97MB per DMA)
STG_CHUNK = 10240  # staging tile width (fp16, 2.5MB per logits DMA)
PS_W = 1024  # PSUM supertile width (2 banks), one copy per supertile
MM_N = 512  # moving free dim per matmul (PSUM bank limit in fp32)

TRACE = False  # set by test.py for profiling runs
_LAST_RESULTS = {}  # test.py reads exec_time_ns etc. from here


def _install_ntff_hook():
    """Register the axon NTFF profiling hook (antenv.axon_hooks is a stub in
    this container).  Harmless if the .so lacks the profiling symbols."""
    try:
        import antenv

        if getattr(antenv, "axon_hooks", None) is not None:
            return
        mod = types.ModuleType("antenv.axon_hooks")
        mod._hook = None
        mod.set_axon_ntff_profile_hook = lambda h: setattr(mod, "_hook", h)
        mod.get_axon_ntff_profile_hook = lambda: mod._hook
        sys.modules["antenv.axon_hooks"] = mod
        antenv.axon_hooks = mod
        from trn_agent_boot.trn_boot import _ntff_profile_via_ctypes

        hook = _ntff_profile_via_ctypes("/opt/axon/libaxon_pjrt.so")
        if hook is not None:
            mod.set_axon_ntff_profile_hook(hook)
    except Exception:
        pass


# ---------------------------------------------------------------- host prep
def _fold(a, cols):
    """[cols*128] -> [128, cols] with column j = slice j*128:(j+1)*128."""
    return np.ascontiguousarray(a.reshape(cols, P).T)


def _levels_for_core(users_c):
    """occ/prev per flat pair index (p = r*S + t, natural order)."""
    occ = np.zeros(N, np.int32)
    prev = np.full(N, -1, np.int32)
    for r in range(R):
        seen_cnt = {}
        seen_last = {}
        row = users_c[r]
        for t in range(S):
            u = int(row[t])
            p = r * S + t
            occ[p] = seen_cnt.get(u, 0)
            prev[p] = seen_last.get(u, -1)
            seen_cnt[u] = occ[p] + 1
            seen_last[u] = p
    return occ, prev


def _build_core_data(users, items, h0, with_h0):
    """Per-core level structure + global padded sizes."""
    cores = []
    kmax = 1
    for c in range(NC):
        occ, prev = _levels_for_core(users[c * R : (c + 1) * R])
        cores.append((occ, prev))
        kmax = max(kmax, int(occ.max()) + 1)

    nk = [0] * kmax
    for occ, _ in cores:
        for k in range(1, kmax):
            nk[k] = max(nk[k], int((occ == k).sum()))
    nk = [max(2, n) if k > 0 else 0 for k, n in enumerate(nk)]

    per_core = []
    for c in range(NC):
        occ, prev = cores[c]
        items_c = items[c * R : (c + 1) * R].reshape(-1).astype(np.int32)
        d = {"items_all": _fold(items_c, NCH)}
        if with_h0:
            users_c = users[c * R : (c + 1) * R].reshape(-1).astype(np.int32)
            local_r = np.repeat(np.arange(R, dtype=np.int32), S)
            d["h0_idx"] = _fold(local_r * U + users_c, NCH)
            d["h0c"] = np.ascontiguousarray(
                h0[c * R : (c + 1) * R].reshape(R * U, H), dtype=np.float32
            )
        for k in range(1, kmax):
            n = nk[k]
            J = (n + P - 1) // P
            pk = np.nonzero(occ == k)[0]
            prev_v = np.full(n, -1.0, np.float32)
            pk_v = np.full(J * P, -1.0, np.float32)
            idx_v = np.zeros(J * P, np.int32)
            invm = np.ones(N, np.float32)
            m = len(pk)
            prev_v[:m] = prev[pk]
            pk_v[:m] = pk
            idx_v[:m] = items_c[pk]
            invm[pk] = 0.0
            # prev indices replicated across partitions (comparand for is_equal)
            d[f"prev{k}"] = np.ascontiguousarray(
                np.broadcast_to(prev_v[None, :], (P, n))
            )
            if k > 1:
                # compact index of prev within level k-1's pair list
                prev_pk = np.nonzero(occ == k - 1)[0]
                pos = {int(p): i for i, p in enumerate(prev_pk)}
                ci = np.full(n, -1.0, np.float32)
                for i, p in enumerate(pk):
                    ci[i] = pos[int(prev[p])]
                d[f"prevci{k}"] = np.ascontiguousarray(
                    np.broadcast_to(ci[None, :], (P, n))
                )
            d[f"pk{k}"] = _fold(pk_v, J)
            d[f"idx{k}"] = _fold(idx_v, J)
            d[f"invm{k}"] = _fold(invm, NCH)
        per_core.append(d)
    return per_core, kmax, nk


# ---------------------------------------------------------------- device build
def _build_program(kmax, nk, with_h0):
    import concourse.bacc as bacc
    import concourse.mybir as mybir
    import concourse.tile as tile
    from concourse import bass

    f32 = mybir.dt.float32
    f16 = mybir.dt.float16
    i32 = mybir.dt.int32
    AF = mybir.ActivationFunctionType
    OP = mybir.AluOpType

    nc = bacc.Bacc(None, target_bir_lowering=False)

    # ---- DRAM I/O
    items_all = nc.dram_tensor("items_all", [P, NCH], i32, kind="ExternalInput")
    P_cat = nc.dram_tensor("P_cat", [V, H2 + H], f32, kind="ExternalInput")
    W_ru = nc.dram_tensor("W_ru", [H, H2], f32, kind="ExternalInput")
    W_c = nc.dram_tensor("W_c", [H, H], f32, kind="ExternalInput")
    ws = nc.dram_tensor("ws", [H, VP], f16, kind="ExternalInput")
    logits = nc.dram_tensor("logits", [N, VP], f16, kind="ExternalOutput")
    lvl_in = {}
    for k in range(1, kmax):
        n = nk[k]
        J = (n + P - 1) // P
        lvl_in[k] = dict(
            prev=nc.dram_tensor(f"prev{k}", [P, n], f32, kind="ExternalInput"),
            pk=nc.dram_tensor(f"pk{k}", [P, J], f32, kind="ExternalInput"),
            idx=nc.dram_tensor(f"idx{k}", [P, J], i32, kind="ExternalInput"),
            invm=nc.dram_tensor(f"invm{k}", [P, NCH], f32, kind="ExternalInput"),
        )
        if k > 1:
            lvl_in[k]["prevci"] = nc.dram_tensor(
                f"prevci{k}", [P, n], f32, kind="ExternalInput"
            )
    if with_h0:
        h0_idx = nc.dram_tensor("h0_idx", [P, NCH], i32, kind="ExternalInput")
        h0c = nc.dram_tensor("h0c", [R * U, H], f32, kind="ExternalInput")

    ws_splits = [(v0, min(WS_CHUNK, VP - v0)) for v0 in range(0, VP, WS_CHUNK)]

    with tile.TileContext(nc) as tc, tc.tile_pool(name="const", bufs=1) as cpool:
        ws_sb = []
        with (
            tc.tile_pool(name="scan", bufs=2) as spool,
            tc.tile_pool(name="scan_ps", bufs=1, space="PSUM") as spsum,
        ):
            # ---- sync queue: scan-critical loads FIRST, bulk ws LAST
            items_sb = cpool.tile([P, NCH], i32, tag="items_sb")
            nc.sync.dma_start(items_sb[:], items_all[:])
            lvl_sb = {}
            for k in range(1, kmax):
                io = lvl_in[k]
                n = nk[k]
                J = (n + P - 1) // P
                invm_sb = spool.tile([P, NCH], f32, tag="invm_sb", bufs=kmax, name="invm_sb")
                nc.sync.dma_start(invm_sb[:], io["invm"][:])
                idx_sb = spool.tile([P, J], i32, tag="idx_sb", bufs=kmax, name="idx_sb")
                nc.sync.dma_start(idx_sb[:], io["idx"][:])
                prev_sb = spool.tile([P, n], f32, tag="prev_sb", bufs=kmax, name="prev_sb")
                nc.sync.dma_start(prev_sb[:], io["prev"][:])
                pk_sb = spool.tile([P, J], f32, tag="pk_sb", bufs=kmax, name="pk_sb")
                nc.sync.dma_start(pk_sb[:], io["pk"][:])
                prevci_sb = None
                if k > 1:
                    prevci_sb = spool.tile(
                        [P, n], f32, tag="prevci_sb", bufs=kmax, name="prevci_sb"
                    )
                    nc.sync.dma_start(prevci_sb[:], io["prevci"][:])
                lvl_sb[k] = (invm_sb, idx_sb, prev_sb, pk_sb, prevci_sb)
            if with_h0:
                h0_idx_sb = cpool.tile([P, NCH], i32, tag="h0_idx_sb")
                nc.sync.dma_start(h0_idx_sb[:], h0_idx[:])
            w_ru_sb = cpool.tile([H, H2], f32, tag="w_ru")
            nc.sync.dma_start(w_ru_sb[:], W_ru[:])
            w_c_sb = cpool.tile([H, H], f32, tag="w_c")
            nc.sync.dma_start(w_c_sb[:], W_c[:])
            # bulk ws load LAST on the sync queue
            for i, (v0, w) in enumerate(ws_splits):
                t = cpool.tile([H, w], f16, tag=f"ws{i}", name=f"ws{i}")
                nc.sync.dma_start(t[:], ws[:, v0 : v0 + w])
                ws_sb.append(t)

            # ---- gpsimd queue: tiny iotas first, then gathers
            iota_col_i = cpool.tile([P, NCH], i32, tag="iota_col_i")
            nc.gpsimd.iota(
                iota_col_i[:], pattern=[[P, NCH]], base=0, channel_multiplier=1
            )
            iota_row_i = cpool.tile([P, N], i32, tag="iota_row_i")
            nc.gpsimd.iota(
                iota_row_i[:], pattern=[[1, N]], base=0, channel_multiplier=0
            )
            # DVE copies + identity built on DVE (no gpsimd make_identity)
            iota_col = cpool.tile([P, NCH], f32, tag="iota_col")
            nc.vector.tensor_copy(iota_col[:], iota_col_i[:])
            iota_row = cpool.tile([P, N], f32, tag="iota_row")
            nc.vector.tensor_copy(iota_row[:], iota_row_i[:])
            ident = cpool.tile([P, P], f32, tag="ident")
            nc.vector.tensor_scalar(
                out=ident[:],
                in0=iota_row[:, 0:P],
                scalar1=iota_col[:, 0:1],
                scalar2=None,
                op0=OP.is_equal,
            )

            # L0 embedding gathers head the gpsimd queue after iotas
            g_cat = []
            for c in range(NCH):
                t = spool.tile([P, H2 + H], f32, tag="g_cat", bufs=NCH, name="g_cat")
                nc.gpsimd.indirect_dma_start(
                    out=t[:],
                    out_offset=None,
                    in_=P_cat[:],
                    in_offset=bass.IndirectOffsetOnAxis(
                        ap=items_sb[:, c : c + 1], axis=0
                    ),
                )
                g_cat.append(t)
            if with_h0:
                g_h0 = []
                for c in range(NCH):
                    g = spool.tile([P, H], f32, tag="g_h0", bufs=NCH, name="g_h0")
                    nc.gpsimd.indirect_dma_start(
                        out=g[:],
                        out_offset=None,
                        in_=h0c[:],
                        in_offset=bass.IndirectOffsetOnAxis(
                            ap=h0_idx_sb[:, c : c + 1], axis=0
                        ),
                    )
                    g_h0.append(g)
            # per-level embedding gathers (prefetched, level-major)
            lvl_emb = {}
            for k in range(1, kmax):
                n = nk[k]
                J = (n + P - 1) // P
                idx_sb = lvl_sb[k][1]
                embs = []
                for j in range(J):
                    j0 = j * P
                    nj = min(P, n - j0)
                    e_cat = spool.tile(
                        [P, H2 + H], f32, tag="e_cat", bufs=2 * kmax, name="e_cat"
                    )
                    nc.gpsimd.indirect_dma_start(
                        out=e_cat[:nj, :],
                        out_offset=None,
                        in_=P_cat[:],
                        in_offset=bass.IndirectOffsetOnAxis(
                            ap=idx_sb[:nj, j : j + 1], axis=0
                        ),
                    )
                    embs.append(e_cat)
                lvl_emb[k] = embs

            # persistent state
            h_nat = [
                cpool.tile([P, H], f32, tag=f"h_nat{c}", name=f"h_nat{c}")
                for c in range(NCH)
            ]
            hT = [
                cpool.tile([H, P], f16, tag=f"hT{c}", name=f"hT{c}")
                for c in range(NCH)
            ]

            # ---------- level 0: all 512 pairs, full width, transposed layout
            zT = cpool.tile([H, N], f32, tag="zT")
            cT = cpool.tile([H, N], f32, tag="cT")
            hT0 = cpool.tile([H, N], f32, tag="hT0")
            z_ps = spsum.tile([H, N], f32, tag="z_ps2", name="z_ps")
            c_ps = spsum.tile([H, N], f32, tag="c_ps2", name="c_ps")

            if not with_h0:
                for c in range(NCH):
                    nc.tensor.matmul(
                        z_ps[:, c * P : (c + 1) * P],
                        g_cat[c][:, H:H2],
                        ident[:],
                        is_transpose=True,
                        start=(c == 0),
                        stop=(c == NCH - 1),
                    )
                    nc.tensor.matmul(
                        c_ps[:, c * P : (c + 1) * P],
                        g_cat[c][:, H2 : H2 + H],
                        ident[:],
                        is_transpose=True,
                        start=(c == 0),
                        stop=(c == NCH - 1),
                    )
                nc.scalar.activation(zT[:], z_ps[:], AF.Sigmoid)
                nc.scalar.activation(cT[:], c_ps[:], AF.Tanh)
                # h = (1-z)*c = c - z*c
                nc.vector.tensor_mul(hT0[:], zT[:], cT[:])
                nc.vector.tensor_sub(hT0[:], cT[:], hT0[:])
            else:
                hp_ps = spsum.tile([H, N], f32, tag="hp_ps", name="hp_ps")
                for c in range(NCH):
                    nc.tensor.matmul(
                        hp_ps[:, c * P : (c + 1) * P],
                        g_h0[c][:],
                        ident[:],
                        is_transpose=True,
                        start=(c == 0),
                        stop=(c == NCH - 1),
                    )
                hprevT = cpool.tile([H, N], f32, tag="hprevT0")
                nc.vector.tensor_copy(hprevT[:], hp_ps[:])

                r_ps = spsum.tile([H, N], f32, tag="r_ps", name="r_ps")
                for c in range(NCH):
                    nc.tensor.matmul(
                        r_ps[:, c * P : (c + 1) * P],
                        g_cat[c][:, 0:H],
                        ident[:],
                        is_transpose=True,
                        start=(c == 0),
                        stop=False,
                    )
                    nc.tensor.matmul(
                        z_ps[:, c * P : (c + 1) * P],
                        g_cat[c][:, H:H2],
                        ident[:],
                        is_transpose=True,
                        start=(c == 0),
                        stop=False,
                    )
                nc.tensor.matmul(
                    r_ps[:], w_ru_sb[:, 0:H], hprevT[:], start=False, stop=True
                )
                nc.tensor.matmul(
                    z_ps[:], w_ru_sb[:, H:H2], hprevT[:], start=False, stop=True
                )
                rT = cpool.tile([H, N], f32, tag="rT0")
                nc.scalar.activation(rT[:], r_ps[:], AF.Sigmoid)
                nc.scalar.activation(zT[:], z_ps[:], AF.Sigmoid)
                rh = cpool.tile([H, N], f32, tag="rh0")
                nc.vector.tensor_mul(rh[:], rT[:], hprevT[:])
                for c in range(NCH):
                    nc.tensor.matmul(
                        c_ps[:, c * P : (c + 1) * P],
                        g_cat[c][:, H2 : H2 + H],
                        ident[:],
                        is_transpose=True,
                        start=(c == 0),
                        stop=False,
                    )
                nc.tensor.matmul(c_ps[:], w_c_sb[:], rh[:], start=False, stop=True)
                nc.scalar.activation(cT[:], c_ps[:], AF.Tanh)
                # h = c + z*(hprev - c)
                nc.vector.tensor_sub(hT0[:], hprevT[:], cT[:])
                nc.vector.tensor_mul(hT0[:], zT[:], hT0[:])
                nc.vector.tensor_add(hT0[:], cT[:], hT0[:])

            # h_nat chunks (natural layout) from hT0
            for c in range(NCH):
                ps = spsum.tile([P, P], f32, tag="tr_ps", bufs=1, name="tr_ps")
                nc.tensor.transpose(ps[:], hT0[:, c * P : (c + 1) * P], ident[:])
                nc.vector.tensor_copy(h_nat[c][:], ps[:])

            # ---------- levels 1..kmax-1 (compact, padded size nk[k])
            hnew_prev = None
            for k in range(1, kmax):
                n = nk[k]
                J = (n + P - 1) // P
                invm_sb, _, prev_sb, pk_sb, prevci_sb = lvl_sb[k]
                Jp = (nk[k - 1] + P - 1) // P if k > 1 else NCH

                # one-hot gather/scatter matrices built just-in-time (DVE)
                sgs = {}
                sss = {}
                for j in range(J):
                    j0 = j * P
                    nj = min(P, n - j0)
                    for c in range(Jp):
                        sg_c = spool.tile(
                            [P, nj], f32, tag="sg_c", bufs=2 * NCH, name="sg_c"
                        )
                        src_in = prev_sb if k == 1 else prevci_sb
                        nc.vector.tensor_scalar(
                            out=sg_c[:],
                            in0=src_in[:, j0 : j0 + nj],
                            scalar1=iota_col[:, c : c + 1],
                            scalar2=None,
                            op0=OP.is_equal,
                        )
                        sgs[(j, c)] = sg_c
                    for c in range(NCH):
                        ss_c = spool.tile(
                            [P, P], f32, tag="ss_c", bufs=2 * NCH, name="ss_c"
                        )
                        nc.vector.tensor_scalar(
                            out=ss_c[:nj, :],
                            in0=iota_row[:nj, c * P : (c + 1) * P],
                            scalar1=pk_sb[:nj, j : j + 1],
                            scalar2=None,
                            op0=OP.is_equal,
                        )
                        sss[(j, c)] = ss_c

                hnew_nat = []
                for j in range(J):
                    j0 = j * P
                    nj = min(P, n - j0)
                    e_cat = lvl_emb[k][j]
                    # gather h_prev directly in transposed layout [H, nj]
                    hp_ps = spsum.tile([H, P], f32, tag="hp_ps", name="hp_ps")
                    if k == 1:
                        for c in range(NCH):
                            nc.tensor.matmul(
                                hp_ps[:, :nj],
                                h_nat[c][:],
                                sgs[(j, c)][:],
                                start=(c == 0),
                                stop=(c == NCH - 1),
                            )
                    else:
                        for ji, (hnp, njp, _) in enumerate(hnew_prev):
                            nc.tensor.matmul(
                                hp_ps[:, :nj],
                                hnp[:njp, :],
                                sgs[(j, ji)][:njp, :],
                                start=(ji == 0),
                                stop=(ji == len(hnew_prev) - 1),
                            )
                    hprevT = spool.tile([H, P], f32, tag="hprevT", name="hprevT")
                    nc.vector.tensor_copy(hprevT[:, :nj], hp_ps[:, :nj])

                    # GRU math; embedding rows enter via transpose-matmuls
                    r_ps = spsum.tile([H, P], f32, tag="r_ps", name="r_ps")
                    nc.tensor.matmul(
                        r_ps[:, :nj],
                        e_cat[:nj, 0:H],
                        ident[:nj, :nj],
                        is_transpose=True,
                        start=True,
                        stop=False,
                    )
                    nc.tensor.matmul(
                        r_ps[:, :nj],
                        w_ru_sb[:, 0:H],
                        hprevT[:, :nj],
                        start=False,
                        stop=True,
                    )
                    rT = spool.tile([H, P], f32, tag="rT_l", name="rT")
                    nc.scalar.activation(rT[:, :nj], r_ps[:, :nj], AF.Sigmoid)
                    z_ps2 = spsum.tile([H, P], f32, tag="z_ps2", name="z_ps2")
                    nc.tensor.matmul(
                        z_ps2[:, :nj],
                        e_cat[:nj, H:H2],
                        ident[:nj, :nj],
                        is_transpose=True,
                        start=True,
                        stop=False,
                    )
                    nc.tensor.matmul(
                        z_ps2[:, :nj],
                        w_ru_sb[:, H:H2],
                        hprevT[:, :nj],
                        start=False,
                        stop=True,
                    )
                    zTl = spool.tile([H, P], f32, tag="zT_l", name="zTl")
                    nc.scalar.activation(zTl[:, :nj], z_ps2[:, :nj], AF.Sigmoid)
                    rh = spool.tile([H, P], f32, tag="rh_l", name="rh")
                    nc.vector.tensor_mul(rh[:, :nj], rT[:, :nj], hprevT[:, :nj])
                    c_ps2 = spsum.tile([H, P], f32, tag="c_ps2", name="c_ps2")
                    nc.tensor.matmul(
                        c_ps2[:, :nj],
                        e_cat[:nj, H2 : H2 + H],
                        ident[:nj, :nj],
                        is_transpose=True,
                        start=True,
                        stop=False,
                    )
                    nc.tensor.matmul(
                        c_ps2[:, :nj],
                        w_c_sb[:],
                        rh[:, :nj],
                        start=False,
                        stop=True,
                    )
                    cTl = spool.tile([H, P], f32, tag="cT_l", name="cTl")
                    nc.scalar.activation(cTl[:, :nj], c_ps2[:, :nj], AF.Tanh)
                    # h_new = c + z*(hprev - c)
                    hnT = spool.tile([H, P], f32, tag="hnT_l", name="hnT")
                    nc.vector.tensor_sub(hnT[:, :nj], hprevT[:, :nj], cTl[:, :nj])
                    nc.vector.tensor_mul(hnT[:, :nj], zTl[:, :nj], hnT[:, :nj])
                    nc.vector.tensor_add(hnT[:, :nj], cTl[:, :nj], hnT[:, :nj])

                    hn = spool.tile([P, H], f32, tag="hn_nat", bufs=6, name="hn")
                    ps = spsum.tile([P, P], f32, tag="tr_ps", bufs=1, name="tr_ps")
                    nc.tensor.transpose(ps[:nj, :H], hnT[:, :nj], ident[:H, :H])
                    nc.vector.tensor_copy(hn[:nj, :], ps[:nj, :H])
                    hnew_nat.append((hn, nj, j0))

                # scatter back into h_nat (masked replace, fused update)
                for c in range(NCH):
                    d_ps = spsum.tile([P, H], f32, tag="d_ps", name="d_ps")
                    for ji, (hn, nj, j0) in enumerate(hnew_nat):
                        nc.tensor.matmul(
                            d_ps[:],
                            sss[(ji, c)][:nj, :],
                            hn[:nj, :],
                            start=(ji == 0),
                            stop=(ji == len(hnew_nat) - 1),
                        )
                    # h_nat = h_nat * invm + delta   (one DVE op)
                    nc.vector.scalar_tensor_tensor(
                        out=h_nat[c][:],
                        in0=h_nat[c][:],
                        scalar=invm_sb[:, c : c + 1],
                        in1=d_ps[:],
                        op0=OP.mult,
                        op1=OP.add,
                    )
                hnew_prev = hnew_nat

            # ---------- final transposed fp16 state for the big matmul
            for c in range(NCH):
                ps = spsum.tile([P, P], f32, tag="tr_ps", bufs=1, name="tr_ps")
                nc.tensor.transpose(ps[:], h_nat[c][:], ident[:])
                nc.vector.tensor_copy(hT[c][:], ps[:])

        # ---------- big projection: logits[128c : 128c+128, :] = hT[c].T @ ws
        with (
            tc.tile_pool(name="big", bufs=3) as bpool,
            tc.tile_pool(name="big_ps", bufs=4, space="PSUM") as bpsum,
        ):
            cp = 0
            for c in range(NCH):
                for v0 in range(0, VP, STG_CHUNK):
                    stage = bpool.tile([P, STG_CHUNK], f16, tag="stage", name="stage")
                    for t0 in range(0, STG_CHUNK, PS_W):
                        o_ps = bpsum.tile([P, PS_W], f32, tag="o_ps", name="o_ps")
                        for m0 in range(0, PS_W, MM_N):
                            wsi, woff = divmod(v0 + t0 + m0, WS_CHUNK)
                            nc.tensor.matmul(
                                o_ps[:, m0 : m0 + MM_N],
                                hT[c][:],
                                ws_sb[wsi][:, woff : woff + MM_N],
                                start=True,
                                stop=True,
                            )
                        # ACT is ~1.2x faster per copy than DVE -> 6:5 split
                        if cp % 11 in (0, 2, 4, 6, 8, 10):
                            nc.scalar.copy(stage[:, t0 : t0 + PS_W], o_ps[:])
                        else:
                            nc.vector.tensor_copy(stage[:, t0 : t0 + PS_W], o_ps[:])
                        cp += 1
                    nc.sync.dma_start(
                        logits[c * P : (c + 1) * P, v0 : v0 + STG_CHUNK],
                        stage[:],
                    )

    nc.finalize()
    return nc


_PROGRAM_CACHE = {}


def kernel(users, items, h0, P_ru, W_ru, b_ru, P_c, W_c, b_c, ws):
    _install_ntff_hook()
    from concourse.bass_utils import run_bass_kernel_spmd

    users = np.asarray(users)
    items = np.asarray(items)
    h0 = np.asarray(h0, dtype=np.float32)
    with_h0 = bool(np.any(h0))

    per_core, kmax, nk = _build_core_data(users, items, h0, with_h0)

    key = (kmax, tuple(nk), with_h0)
    if key not in _PROGRAM_CACHE:
        _PROGRAM_CACHE[key] = _build_program(kmax, nk, with_h0)
    nc = _PROGRAM_CACHE[key]

    # biases folded into the embedding table: activations need no bias input
    P_cat = np.concatenate(
        [
            np.asarray(P_ru, dtype=np.float32)
            + np.asarray(b_ru, dtype=np.float32)[None, :],
            np.asarray(P_c, dtype=np.float32)
            + np.asarray(b_c, dtype=np.float32)[None, :],
        ],
        axis=1,
    )
    ws_pad = np.zeros((H, VP), np.float16)
    ws_pad[:, :V] = np.asarray(ws, dtype=np.float16)
    shared = {
        "P_cat": P_cat,
        "W_ru": np.ascontiguousarray(W_ru, dtype=np.float32),
        "W_c": np.ascontiguousarray(W_c, dtype=np.float32),
        "ws": ws_pad,
    }
    in_maps = [{**shared, **per_core[c]} for c in range(NC)]

    res = run_bass_kernel_spmd(nc, in_maps, core_ids=list(range(NC)), trace=TRACE)
    _LAST_RESULTS["exec_time_ns"] = res.exec_time_ns
    _LAST_RESULTS["mean_exec_time_ns"] = res.mean_exec_time_ns
    _LAST_RESULTS["trace"] = res.instructions_and_trace
    _LAST_RESULTS["profile_json"] = res.profile_json

    out = np.empty((B * S, V), np.float32)
    for c in range(NC):
        out[c * N : (c + 1) * N] = res.results[c]["logits"][:, :V]
    return out
